# revision 2
# baseline (speedup 1.0000x reference)
"""Causal self-attention with T5 relative-position bias, distributed over
8 NeuronCores (batch x head-group parallel).

Problem: x[2,2048,1024] @ w_qkv -> 16-head causal attention with a T5
bucketed relative-position bias added to the scores -> @ w_proj.

Sharding: core c handles batch b = c//4 and heads [4*(c%4), 4*(c%4)+4).
Each core computes a partial output projection (its heads' slice of the
c_proj contraction) in bf16; the host sums the 4 partials per batch in f32.

On-chip dataflow (per core, bf16 matmuls with f32 PSUM accumulation):
  one packed bf16 input tensor, host pre-tiled into the exact SBUF
  layouts (x[b]^T, head-sliced QKV/proj weights with the 1/sqrt(d) scale
  folded into wq, and per-head exp(bias) diagonal-band tables that also
  apply the causal mask via zeros);
  x^T -> Q^T,K^T [128,T] per head pair and V [T,64]-tiles per head;
  per head: scores^T[k,q] = K^T(slice)^T @ Q^T(slice); exp on ScalarE
  (PSUM->bf16); multiply by the exp(bias) band table (VectorE); A@V with
  a ones-column appended to V so the softmax denominator falls out of the
  same matmul; normalize with VectorE reciprocal + partition-broadcast;
  joint projection over both head pairs -> bf16 partial out.
"""
import math
from contextlib import ExitStack

import numpy as np
import ml_dtypes

import concourse.bass as bass
import concourse.bacc as bacc
import concourse.mybir as mybir
import concourse.tile as tile
from concourse.bass_utils import run_bass_kernel_spmd

# Problem constants (hardcoded per contract)
B, T, C, H = 2, 2048, 1024, 16
D = C // H                      # 64
NUM_BUCKETS, MAX_DISTANCE = 32, 2048
N_CORES = 8
HPC = 4                         # heads per core
KT_N = T // 128                 # 16 k-tiles
W_EXPB = 2560                   # diag table width; expb[i,m] = e(m-i-512)

F32 = mybir.dt.float32
BF16 = mybir.dt.bfloat16
EXP = mybir.ActivationFunctionType.Exp

# packed-tensor element offsets (bf16 elements)
OFF_XT = 0                                   # [1024, 2048] x[b].T
OFF_WQ = OFF_XT + C * T                      # [128, 2048] pre-tiled wq*scale
OFF_WK = OFF_WQ + 128 * 2048
OFF_WV = OFF_WK + 128 * 2048
OFF_WP = OFF_WV + 128 * 2048                 # [128, 2048] pre-tiled wp
OFF_EB = OFF_WP + 128 * 2048                 # 4 x [128, 2560] expb blocks
PACK_N = OFF_EB + 4 * 128 * W_EXPB           # 4,456,448 bf16 elements


# ---------------------------------------------------------------- host math
def _bucket_causal(d):
    """T5 causal bucket for distances d>=0.

    Runs the same jnp ops as the reference on the default jax backend so
    that discrete bucket boundaries match the graded reference bit-exactly
    (the trn/axon backend rounds f32->int32 where numpy truncates).
    """
    import jax.numpy as jnp

    rp = jnp.asarray(np.asarray(d, dtype=np.int32))
    max_exact = NUM_BUCKETS // 2
    is_small = rp < max_exact
    rp_safe = jnp.maximum(rp, 1).astype(jnp.float32)
    large = max_exact + (
        jnp.log(rp_safe / max_exact)
        / math.log(MAX_DISTANCE / max_exact)
        * (NUM_BUCKETS - max_exact)
    ).astype(jnp.int32)
    large = jnp.minimum(large, NUM_BUCKETS - 1)
    return np.asarray(jnp.where(is_small, rp, large))


def _expb_tables(rel_table, h0):
    """[HPC, 128, W_EXPB] f32: expb[lh][i, m] = exp(bias(d)) at d = m-i-512,
    zero for d < 0 (applies the causal mask)."""
    j = np.arange(W_EXPB + 127)
    d = j - 639
    valid = d >= 0
    buckets = _bucket_causal(np.where(valid, d, 0))
    out = np.zeros((HPC, 128, W_EXPB), dtype=np.float32)
    i_idx = np.arange(128)[:, None]
    m_idx = np.arange(W_EXPB)[None, :]
    jj = m_idx - i_idx + 127
    for lh in range(HPC):
        vec = np.where(valid, np.exp(rel_table[buckets, h0 + lh]), 0.0).astype(
            np.float32
        )
        out[lh] = vec[jj]
    return out


def _tile_w(w):
    """[1024, 256] -> [128, 2048] with row p = concat_ct(w[128*ct + p, :])."""
    return np.ascontiguousarray(
        w.reshape(8, 128, 256).transpose(1, 0, 2).reshape(128, 2048)
    )


def host_in_maps(x, w_qkv, w_proj, rel_table):
    """Build the 8 per-core input maps: one packed bf16 tensor each."""
    bf16 = ml_dtypes.bfloat16
    x = np.asarray(x, dtype=np.float32)
    w_qkv = np.asarray(w_qkv, dtype=np.float32)
    w_proj = np.asarray(w_proj, dtype=np.float32)
    rel_table = np.asarray(rel_table, dtype=np.float32)
    scale = 1.0 / math.sqrt(D)
    xT = [np.ascontiguousarray(x[b].T).astype(bf16) for b in range(B)]
    in_maps = []
    for c in range(N_CORES):
        b, h0 = c // 4, 4 * (c % 4)
        cs = slice(64 * h0, 64 * h0 + 256)
        pack = np.empty(PACK_N, dtype=bf16)
        pack[OFF_XT:OFF_WQ] = xT[b].reshape(-1)
        pack[OFF_WQ:OFF_WK] = _tile_w(w_qkv[:, cs] * scale).astype(bf16).reshape(-1)
        pack[OFF_WK:OFF_WV] = _tile_w(
            w_qkv[:, 1024 + 64 * h0 : 1024 + 64 * h0 + 256]
        ).astype(bf16).reshape(-1)
        pack[OFF_WV:OFF_WP] = _tile_w(
            w_qkv[:, 2048 + 64 * h0 : 2048 + 64 * h0 + 256]
        ).astype(bf16).reshape(-1)
        # wp [256, 1024] -> [128, 2048], row p = concat(wp[p], wp[128+p])
        wp = w_proj[cs, :]
        pack[OFF_WP:OFF_EB] = (
            wp.reshape(2, 128, 1024).transpose(1, 0, 2).reshape(-1).astype(bf16)
        )
        pack[OFF_EB:] = _expb_tables(rel_table, h0).astype(bf16).reshape(-1)
        in_maps.append({"pack": pack})
    return in_maps


# ------------------------------------------------------------- bass program
def build_program():
    nc = bacc.Bacc("TRN2", target_bir_lowering=False, debug=False)
    PACK = nc.dram_tensor("pack", [PACK_N], BF16, kind="ExternalInput")
    OUT = nc.dram_tensor("out", [T, C], BF16, kind="ExternalOutput")

    def prearr(off, n, w):
        return PACK[off : off + n * w].rearrange("(p w) -> p w", w=w)

    with tile.TileContext(nc) as tc, ExitStack() as ctx:
        persist = ctx.enter_context(tc.tile_pool(name="persist", bufs=1))
        work = ctx.enter_context(tc.tile_pool(name="work", bufs=1))

        # ---- persistent tiles (all bf16)
        QT = [persist.tile([128, T], BF16, tag=f"qt{g}", name=f"qt{g}") for g in range(2)]
        KT = [persist.tile([128, T], BF16, tag=f"kt{g}", name=f"kt{g}") for g in range(2)]
        V = [persist.tile([128, KT_N * 65], BF16, tag=f"v{lh}", name=f"v{lh}") for lh in range(HPC)]
        EB = [persist.tile([128, 2 * W_EXPB], BF16, tag=f"eb{g}", name=f"eb{g}") for g in range(2)]
        YN = [persist.tile([128, T], BF16, tag=f"yn{g}", name=f"yn{g}") for g in range(2)]
        WPS = persist.tile([128, 2048], BF16, tag="wp", name="wp")

        xw = ctx.enter_context(tc.tile_pool(name="xw", bufs=1))
        xt_sb = xw.tile([128, 8 * T], BF16, tag="xt", name="xt")
        wq_sb = xw.tile([128, 2048], BF16, tag="wq", name="wqs")
        wk_sb = xw.tile([128, 2048], BF16, tag="wk", name="wks")
        wv_sb = xw.tile([128, 2048], BF16, tag="wv", name="wvs")

        # ---- input DMAs; xt split in 4 so QKV accumulation can start early
        xt3 = xt_sb[:].rearrange("p (c t) -> p c t", c=8)
        pk3 = PACK[OFF_XT : OFF_XT + C * T].rearrange("(c p t) -> p c t", c=8, t=T)
        nc.sync.dma_start(wq_sb[:], prearr(OFF_WQ, 128, 2048))
        nc.sync.dma_start(wk_sb[:], prearr(OFF_WK, 128, 2048))
        nc.sync.dma_start(wv_sb[:], prearr(OFF_WV, 128, 2048))
        for cq in range(4):
            nc.sync.dma_start(
                xt3[:, 2 * cq : 2 * (cq + 1), :], pk3[:, 2 * cq : 2 * (cq + 1), :]
            )
        for g in range(2):
            for s in range(2):
                nc.sync.dma_start(
                    EB[g][:, W_EXPB * s : W_EXPB * (s + 1)],
                    prearr(OFF_EB + (2 * g + s) * 128 * W_EXPB, 128, W_EXPB),
                )
        nc.sync.dma_start(WPS[:], prearr(OFF_WP, 128, 2048))
        for lh in range(HPC):
            nc.vector.memset(V[lh][:], 1.0)

        # ===== interleaved QKV / attention phasing ============================
        # A1: QT0/KT0 + V(all heads) -> B1: attention pair 0
        # A2: QT1/KT1              -> B2: attention pair 1 -> C: projection
        def qkv_pair(g, ph):
            psA = ph.enter_context(tc.tile_pool(name=f"psA{g}", bufs=4, space="PSUM"))
            for w_sb, dst in ((wq_sb, QT[g]), (wk_sb, KT[g])):
                for n in range(4):
                    ps = psA.tile([128, 512], F32, tag="qkv", name="qkvps")
                    for ct in range(8):
                        nc.tensor.matmul(
                            ps[:],
                            w_sb[:, 256 * ct + 128 * g : 256 * ct + 128 * (g + 1)],
                            xt_sb[:, T * ct + 512 * n : T * ct + 512 * (n + 1)],
                            start=(ct == 0),
                            stop=(ct == 7),
                        )
                    nc.vector.tensor_copy(dst[:, 512 * n : 512 * (n + 1)], ps[:])

        def v_all(ph):
            psV = ph.enter_context(tc.tile_pool(name="psV", bufs=2, space="PSUM"))
            for tt in range(KT_N):
                ps = psV.tile([128, 256], F32, tag="vps", name="vps")
                for ct in range(8):
                    nc.tensor.matmul(
                        ps[:],
                        xt_sb[:, T * ct + 128 * tt : T * ct + 128 * (tt + 1)],
                        wv_sb[:, 256 * ct : 256 * (ct + 1)],
                        start=(ct == 0),
                        stop=(ct == 7),
                    )
                for lh in range(HPC):
                    nc.vector.tensor_copy(
                        V[lh][:, 65 * tt : 65 * tt + 64], ps[:, 64 * lh : 64 * (lh + 1)]
                    )

        def attention_pair(g, ph):
            psS = ph.enter_context(tc.tile_pool(name=f"psS{g}", bufs=1, space="PSUM"))
            psAV = ph.enter_context(tc.tile_pool(name=f"psAV{g}", bufs=1, space="PSUM"))
            eb3 = EB[g][:].rearrange("p (s w) -> p s w", s=2)
            for jc in range(4):       # 512-wide q chunk
                q0 = 512 * jc
                kt_max = (q0 + 511) // 128   # inclusive last k-tile
                # av holds both heads: h0 cols 0:512, h1 cols 512:1024
                av = psAV.tile([128, 1024], F32, tag="av", bufs=2, name="av")
                pend = None     # AV issued one k behind so PE never waits DVE
                for k in range(kt_max + 1):
                    k0 = 128 * k
                    # both heads' scores into one [128, 1024] psum tile
                    ps = psS.tile([128, 1024], F32, tag="s", bufs=2, name="sps")
                    for s in range(2):
                        nc.tensor.matmul(
                            ps[:, 512 * s : 512 * (s + 1)],
                            KT[g][64 * s : 64 * (s + 1), k0 : k0 + 128],
                            QT[g][64 * s : 64 * (s + 1), q0 : q0 + 512],
                            start=True,
                            stop=True,
                        )
                    if pend is not None:
                        _emit_av(g, av, *pend, kt_max)
                    # one exp + one in-place expb-multiply covering both heads;
                    # columns q < k0 are fully causal-masked -> skip them
                    lo2 = max(0, k0 - q0)
                    ps3 = ps[:].rearrange("p (s w) -> p s w", s=2)
                    att = work.tile([128, 1024], BF16, tag="att", bufs=3, name="att")
                    at3 = att[:].rearrange("p (s w) -> p s w", s=2)
                    nc.scalar.activation(at3[:, :, lo2:512], ps3[:, :, lo2:512], EXP)
                    m0 = q0 - k0 + 512
                    nc.vector.tensor_mul(
                        at3[:, :, lo2:512], at3[:, :, lo2:512],
                        eb3[:, :, m0 + lo2 : m0 + 512],
                    )
                    pend = (k, att, lo2)
                _emit_av(g, av, *pend, kt_max)
                # normalize: yn = av[0:64] * (1/av[64]) broadcast
                for s in range(2):
                    rcp = work.tile([1, 512], F32, tag="rcp", bufs=2, name="rcp")
                    nc.vector.reciprocal(rcp[:], av[64:65, 512 * s : 512 * (s + 1)])
                    bc = work.tile([64, 512], F32, tag="bc", bufs=2, name="bc")
                    nc.gpsimd.partition_broadcast(bc[:], rcp[:])
                    nc.vector.tensor_mul(
                        YN[g][64 * s : 64 * (s + 1), q0 : q0 + 512],
                        av[0:64, 512 * s : 512 * (s + 1)],
                        bc[:],
                    )

        def _emit_av(g, av, k, att, lo2, kt_max):
            for s in range(2):
                nc.tensor.matmul(
                    av[0:65, 512 * s + lo2 : 512 * (s + 1)],
                    V[2 * g + s][:, 65 * k : 65 * k + 65],
                    att[:, 512 * s + lo2 : 512 * (s + 1)],
                    start=(k == 0),
                    stop=(k == kt_max),
                )

        with ExitStack() as ph:
            qkv_pair(0, ph)
            v_all(ph)
        with ExitStack() as ph:
            attention_pair(0, ph)
        with ExitStack() as ph:
            qkv_pair(1, ph)
        with ExitStack() as ph:
            attention_pair(1, ph)

        # ======================= phase C: output projection ===================
        with ExitStack() as ph_c:
            psP = ph_c.enter_context(tc.tile_pool(name="psP", bufs=4, space="PSUM"))
            for tt in range(KT_N):
                osb = work.tile([128, C], BF16, tag="osb", bufs=2, name="osb")
                for co in range(2):
                    ps = psP.tile([128, 512], F32, tag="p", name="pps")
                    for g in range(2):
                        nc.tensor.matmul(
                            ps[:],
                            YN[g][:, 128 * tt : 128 * (tt + 1)],
                            WPS[:, 1024 * g + 512 * co : 1024 * g + 512 * (co + 1)],
                            start=(g == 0),
                            stop=(g == 1),
                        )
                    nc.vector.tensor_copy(osb[:, 512 * co : 512 * (co + 1)], ps[:])
                nc.sync.dma_start(OUT[128 * tt : 128 * (tt + 1), :], osb[:])

    nc.compile()
    return nc


_PROGRAM = None


def kernel(x, w_qkv, w_proj, rel_table):
    global _PROGRAM
    if _PROGRAM is None:
        _PROGRAM = build_program()
    in_maps = host_in_maps(x, w_qkv, w_proj, rel_table)
    res = run_bass_kernel_spmd(_PROGRAM, in_maps, core_ids=list(range(N_CORES)))
    out = np.zeros((B, T, C), dtype=np.float32)
    for c in range(N_CORES):
        out[c // 4] += res.results[c]["out"].astype(np.float32)
    return out


# revision 3
# speedup vs baseline: 1.1897x; 1.1897x over previous
"""Causal self-attention with T5 relative-position bias, distributed over
8 NeuronCores (batch x head-group parallel).

Problem: x[2,2048,1024] @ w_qkv -> 16-head causal attention with a T5
bucketed relative-position bias added to the scores -> @ w_proj.

Sharding: core c handles batch b = c//4 and heads [4*(c%4), 4*(c%4)+4).
Each core computes a partial output projection (its heads' slice of the
c_proj contraction) in bf16; the host sums the 4 partials per batch in f32.

On-chip dataflow (per core, bf16 matmuls with f32 PSUM accumulation):
  one packed bf16 input tensor, host pre-tiled into the exact SBUF
  layouts (x[b]^T, head-sliced QKV/proj weights with the 1/sqrt(d) scale
  folded into wq, and per-head exp(bias) diagonal-band tables that also
  apply the causal mask via zeros);
  x^T -> Q^T,K^T [128,T] per head pair and V [T,64]-tiles per head;
  per head: scores^T[k,q] = K^T(slice)^T @ Q^T(slice); exp on ScalarE
  (PSUM->bf16); multiply by the exp(bias) band table (VectorE); A@V with
  a ones-column appended to V so the softmax denominator falls out of the
  same matmul; normalize with VectorE reciprocal + partition-broadcast;
  joint projection over both head pairs -> bf16 partial out.
"""
import math
from contextlib import ExitStack

import numpy as np
import ml_dtypes

import concourse.bass as bass
import concourse.bacc as bacc
import concourse.mybir as mybir
import concourse.tile as tile
from concourse.bass_utils import run_bass_kernel_spmd

# Problem constants (hardcoded per contract)
B, T, C, H = 2, 2048, 1024, 16
D = C // H                      # 64
NUM_BUCKETS, MAX_DISTANCE = 32, 2048
N_CORES = 8
HPC = 4                         # heads per core
KT_N = T // 128                 # 16 k-tiles
W_EXPB = 2560                   # diag table width; expb[i,m] = e(m-i-512)

F32 = mybir.dt.float32
BF16 = mybir.dt.bfloat16
EXP = mybir.ActivationFunctionType.Exp

# packed-tensor element offsets (bf16 elements)
OFF_XT = 0                                   # [1024, 2048] x[b].T
OFF_WQ = OFF_XT + C * T                      # [128, 2048] pre-tiled wq*scale
OFF_WK = OFF_WQ + 128 * 2048
OFF_WV = OFF_WK + 128 * 2048
OFF_WP = OFF_WV + 128 * 2048                 # [128, 2048] pre-tiled wp
OFF_EB = OFF_WP + 128 * 2048                 # 4 x [128, 2560] expb blocks
PACK_N = OFF_EB + 4 * 128 * W_EXPB           # 4,456,448 bf16 elements


# ---------------------------------------------------------------- host math
def _bucket_causal(d):
    """T5 causal bucket for distances d>=0.

    Runs the same jnp ops as the reference on the default jax backend so
    that discrete bucket boundaries match the graded reference bit-exactly
    (the trn/axon backend rounds f32->int32 where numpy truncates).
    """
    import jax.numpy as jnp

    rp = jnp.asarray(np.asarray(d, dtype=np.int32))
    max_exact = NUM_BUCKETS // 2
    is_small = rp < max_exact
    rp_safe = jnp.maximum(rp, 1).astype(jnp.float32)
    large = max_exact + (
        jnp.log(rp_safe / max_exact)
        / math.log(MAX_DISTANCE / max_exact)
        * (NUM_BUCKETS - max_exact)
    ).astype(jnp.int32)
    large = jnp.minimum(large, NUM_BUCKETS - 1)
    return np.asarray(jnp.where(is_small, rp, large))


def _expb_tables(rel_table, h0):
    """[HPC, 128, W_EXPB] f32: expb[lh][i, m] = exp(bias(d)) at d = m-i-512,
    zero for d < 0 (applies the causal mask)."""
    j = np.arange(W_EXPB + 127)
    d = j - 639
    valid = d >= 0
    buckets = _bucket_causal(np.where(valid, d, 0))
    out = np.zeros((HPC, 128, W_EXPB), dtype=np.float32)
    i_idx = np.arange(128)[:, None]
    m_idx = np.arange(W_EXPB)[None, :]
    jj = m_idx - i_idx + 127
    for lh in range(HPC):
        vec = np.where(valid, np.exp(rel_table[buckets, h0 + lh]), 0.0).astype(
            np.float32
        )
        out[lh] = vec[jj]
    return out


def _tile_w(w):
    """[1024, 256] -> [128, 2048] with row p = concat_ct(w[128*ct + p, :])."""
    return np.ascontiguousarray(
        w.reshape(8, 128, 256).transpose(1, 0, 2).reshape(128, 2048)
    )


def host_in_maps(x, w_qkv, w_proj, rel_table):
    """Build the 8 per-core input maps: one packed bf16 tensor each."""
    bf16 = ml_dtypes.bfloat16
    x = np.asarray(x, dtype=np.float32)
    w_qkv = np.asarray(w_qkv, dtype=np.float32)
    w_proj = np.asarray(w_proj, dtype=np.float32)
    rel_table = np.asarray(rel_table, dtype=np.float32)
    scale = 1.0 / math.sqrt(D)
    xT = [np.ascontiguousarray(x[b].T).astype(bf16) for b in range(B)]
    in_maps = []
    for c in range(N_CORES):
        b, h0 = c // 4, 4 * (c % 4)
        cs = slice(64 * h0, 64 * h0 + 256)
        pack = np.empty(PACK_N, dtype=bf16)
        pack[OFF_XT:OFF_WQ] = xT[b].reshape(-1)
        pack[OFF_WQ:OFF_WK] = _tile_w(w_qkv[:, cs] * scale).astype(bf16).reshape(-1)
        pack[OFF_WK:OFF_WV] = _tile_w(
            w_qkv[:, 1024 + 64 * h0 : 1024 + 64 * h0 + 256]
        ).astype(bf16).reshape(-1)
        pack[OFF_WV:OFF_WP] = _tile_w(
            w_qkv[:, 2048 + 64 * h0 : 2048 + 64 * h0 + 256]
        ).astype(bf16).reshape(-1)
        # wp [256, 1024] -> [128, 2048], row p = concat(wp[p], wp[128+p])
        wp = w_proj[cs, :]
        pack[OFF_WP:OFF_EB] = (
            wp.reshape(2, 128, 1024).transpose(1, 0, 2).reshape(-1).astype(bf16)
        )
        pack[OFF_EB:] = _expb_tables(rel_table, h0).astype(bf16).reshape(-1)
        in_maps.append({"pack": pack})
    return in_maps


# ------------------------------------------------------------- bass program
def build_program():
    nc = bacc.Bacc("TRN2", target_bir_lowering=False, debug=False)
    PACK = nc.dram_tensor("pack", [PACK_N], BF16, kind="ExternalInput")
    OUT = nc.dram_tensor("out", [T, C], BF16, kind="ExternalOutput")

    def prearr(off, n, w):
        return PACK[off : off + n * w].rearrange("(p w) -> p w", w=w)

    with tile.TileContext(nc) as tc, ExitStack() as ctx:
        persist = ctx.enter_context(tc.tile_pool(name="persist", bufs=1))
        work = ctx.enter_context(tc.tile_pool(name="work", bufs=1))

        # ---- persistent tiles (all bf16)
        QT = [persist.tile([128, T], BF16, tag=f"qt{g}", name=f"qt{g}") for g in range(2)]
        KT = [persist.tile([128, T], BF16, tag=f"kt{g}", name=f"kt{g}") for g in range(2)]
        V = [persist.tile([128, KT_N * 65], BF16, tag=f"v{lh}", name=f"v{lh}") for lh in range(HPC)]
        EB = [persist.tile([128, 2 * W_EXPB], BF16, tag=f"eb{g}", name=f"eb{g}") for g in range(2)]
        YN = [persist.tile([128, T], BF16, tag=f"yn{g}", name=f"yn{g}") for g in range(2)]
        WPS = persist.tile([128, 2048], BF16, tag="wp", name="wp")

        xw = ctx.enter_context(tc.tile_pool(name="xw", bufs=1))
        xt_sb = xw.tile([128, 8 * T], BF16, tag="xt", name="xt")
        wq_sb = xw.tile([128, 2048], BF16, tag="wq", name="wqs")
        wk_sb = xw.tile([128, 2048], BF16, tag="wk", name="wks")
        wv_sb = xw.tile([128, 2048], BF16, tag="wv", name="wvs")

        # ---- input DMAs; xt split in 4 so QKV accumulation can start early
        xt3 = xt_sb[:].rearrange("p (c t) -> p c t", c=8)
        pk3 = PACK[OFF_XT : OFF_XT + C * T].rearrange("(c p t) -> p c t", c=8, t=T)
        nc.sync.dma_start(wq_sb[:], prearr(OFF_WQ, 128, 2048))
        nc.sync.dma_start(wk_sb[:], prearr(OFF_WK, 128, 2048))
        for cq in range(4):
            nc.sync.dma_start(
                xt3[:, 2 * cq : 2 * (cq + 1), :], pk3[:, 2 * cq : 2 * (cq + 1), :]
            )
        nc.sync.dma_start(wv_sb[:], prearr(OFF_WV, 128, 2048))
        for g in range(2):
            for s in range(2):
                nc.sync.dma_start(
                    EB[g][:, W_EXPB * s : W_EXPB * (s + 1)],
                    prearr(OFF_EB + (2 * g + s) * 128 * W_EXPB, 128, W_EXPB),
                )
        nc.sync.dma_start(WPS[:], prearr(OFF_WP, 128, 2048))
        for lh in range(HPC):
            nc.vector.memset(V[lh][:], 1.0)

        # ===== interleaved QKV / attention phasing ============================
        # A1: QT0/KT0 + V(all heads) -> B1: attention pair 0
        # A2: QT1/KT1              -> B2: attention pair 1 -> C: projection
        def qkv_pair(g, ph):
            psA = ph.enter_context(tc.tile_pool(name=f"psA{g}", bufs=4, space="PSUM"))
            for w_sb, dst in ((wq_sb, QT[g]), (wk_sb, KT[g])):
                for n in range(4):
                    ps = psA.tile([128, 512], F32, tag="qkv", name="qkvps")
                    for ct in range(8):
                        nc.tensor.matmul(
                            ps[:],
                            w_sb[:, 256 * ct + 128 * g : 256 * ct + 128 * (g + 1)],
                            xt_sb[:, T * ct + 512 * n : T * ct + 512 * (n + 1)],
                            start=(ct == 0),
                            stop=(ct == 7),
                        )
                    nc.vector.tensor_copy(dst[:, 512 * n : 512 * (n + 1)], ps[:])

        def v_all(ph):
            psV = ph.enter_context(tc.tile_pool(name="psV", bufs=2, space="PSUM"))
            for tt in range(KT_N):
                ps = psV.tile([128, 256], F32, tag="vps", name="vps")
                for ct in range(8):
                    nc.tensor.matmul(
                        ps[:],
                        xt_sb[:, T * ct + 128 * tt : T * ct + 128 * (tt + 1)],
                        wv_sb[:, 256 * ct : 256 * (ct + 1)],
                        start=(ct == 0),
                        stop=(ct == 7),
                    )
                for lh in range(HPC):
                    nc.vector.tensor_copy(
                        V[lh][:, 65 * tt : 65 * tt + 64], ps[:, 64 * lh : 64 * (lh + 1)]
                    )

        def attention_pair(g, ph):
            psS = ph.enter_context(tc.tile_pool(name=f"psS{g}", bufs=1, space="PSUM"))
            psAV = ph.enter_context(tc.tile_pool(name=f"psAV{g}", bufs=1, space="PSUM"))
            eb3 = EB[g][:].rearrange("p (s w) -> p s w", s=2)
            for jc in range(4):       # 512-wide q chunk
                q0 = 512 * jc
                kt_max = (q0 + 511) // 128   # inclusive last k-tile
                # av holds both heads: h0 cols 0:512, h1 cols 512:1024
                av = psAV.tile([128, 1024], F32, tag="av", bufs=2, name="av")
                pend = None     # AV issued one k behind so PE never waits DVE
                for k in range(kt_max + 1):
                    k0 = 128 * k
                    # both heads' scores into one [128, 1024] psum tile
                    ps = psS.tile([128, 1024], F32, tag="s", bufs=2, name="sps")
                    for s in range(2):
                        nc.tensor.matmul(
                            ps[:, 512 * s : 512 * (s + 1)],
                            KT[g][64 * s : 64 * (s + 1), k0 : k0 + 128],
                            QT[g][64 * s : 64 * (s + 1), q0 : q0 + 512],
                            start=True,
                            stop=True,
                        )
                    if pend is not None:
                        _emit_av(g, av, *pend, kt_max)
                    # one exp + one in-place expb-multiply covering both heads;
                    # columns q < k0 are fully causal-masked -> skip them
                    lo2 = max(0, k0 - q0)
                    ps3 = ps[:].rearrange("p (s w) -> p s w", s=2)
                    att = work.tile([128, 1024], BF16, tag="att", bufs=3, name="att")
                    at3 = att[:].rearrange("p (s w) -> p s w", s=2)
                    nc.scalar.activation(at3[:, :, lo2:512], ps3[:, :, lo2:512], EXP)
                    m0 = q0 - k0 + 512
                    nc.vector.tensor_mul(
                        at3[:, :, lo2:512], at3[:, :, lo2:512],
                        eb3[:, :, m0 + lo2 : m0 + 512],
                    )
                    pend = (k, att, lo2)
                _emit_av(g, av, *pend, kt_max)
                # normalize: yn = av[0:64] * (1/av[64]) broadcast
                for s in range(2):
                    rcp = work.tile([1, 512], F32, tag="rcp", bufs=2, name="rcp")
                    nc.vector.reciprocal(rcp[:], av[64:65, 512 * s : 512 * (s + 1)])
                    bc = work.tile([64, 512], F32, tag="bc", bufs=2, name="bc")
                    nc.gpsimd.partition_broadcast(bc[:], rcp[:])
                    nc.vector.tensor_mul(
                        YN[g][64 * s : 64 * (s + 1), q0 : q0 + 512],
                        av[0:64, 512 * s : 512 * (s + 1)],
                        bc[:],
                    )

        def _emit_av(g, av, k, att, lo2, kt_max):
            for s in range(2):
                nc.tensor.matmul(
                    av[0:65, 512 * s + lo2 : 512 * (s + 1)],
                    V[2 * g + s][:, 65 * k : 65 * k + 65],
                    att[:, 512 * s + lo2 : 512 * (s + 1)],
                    start=(k == 0),
                    stop=(k == kt_max),
                )

        with ExitStack() as ph:
            qkv_pair(0, ph)
            v_all(ph)
        with ExitStack() as ph:
            attention_pair(0, ph)
        with ExitStack() as ph:
            qkv_pair(1, ph)
        with ExitStack() as ph:
            attention_pair(1, ph)

        # ======================= phase C: output projection ===================
        with ExitStack() as ph_c:
            psP = ph_c.enter_context(tc.tile_pool(name="psP", bufs=4, space="PSUM"))
            for tt in range(KT_N):
                osb = work.tile([128, C], BF16, tag="osb", bufs=2, name="osb")
                for co in range(2):
                    ps = psP.tile([128, 512], F32, tag="p", name="pps")
                    for g in range(2):
                        nc.tensor.matmul(
                            ps[:],
                            YN[g][:, 128 * tt : 128 * (tt + 1)],
                            WPS[:, 1024 * g + 512 * co : 1024 * g + 512 * (co + 1)],
                            start=(g == 0),
                            stop=(g == 1),
                        )
                    nc.vector.tensor_copy(osb[:, 512 * co : 512 * (co + 1)], ps[:])
                nc.sync.dma_start(OUT[128 * tt : 128 * (tt + 1), :], osb[:])

    nc.compile()
    return nc


_PROGRAM = None


def kernel(x, w_qkv, w_proj, rel_table):
    global _PROGRAM
    if _PROGRAM is None:
        _PROGRAM = build_program()
    in_maps = host_in_maps(x, w_qkv, w_proj, rel_table)
    res = run_bass_kernel_spmd(_PROGRAM, in_maps, core_ids=list(range(N_CORES)))
    out = np.zeros((B, T, C), dtype=np.float32)
    for c in range(N_CORES):
        out[c // 4] += res.results[c]["out"].astype(np.float32)
    return out
